# revision 25
# baseline (speedup 1.0000x reference)
"""Trainium2 Bass kernel for nn_DatTransformer (sparse hard-max attention).

Sharding: 8 cores = 4 batches x 2 query-halves. Each core holds full K for its
batch (keys in rolled query-half-first order).

Numerics v2: pure f32r (12-bit mantissa) matmuls everywhere -- no fp8
DoubleRow correction. Score error is bounded (~0.06 max on this data); rows
whose device top-2 margin is below TAU get recomputed exactly on the host
from the returned top-8 values/indices (~200 rows of 16384). The selection
threshold (0.95) is >100 away from every row max, so sel is always true and
is also verified host-side from the exported maxima.

  proj:   (wh f32r)^T (xh*2^13 f32r) -> psum = proj*2^13; Act w/ scale+bias
          writes f32r q*2^12 (Q) / k (K) directly.
  scores: (qh*2^12 f32r)^T (kh f32r) -> psum = score*2^12; Act copy w/ scale
          2^-12 -> SBUF scores; DVE max (top-8) + max_index -> winner+margin.
  out:    indirect-gather x[winner] (bf16) -> transpose -> @ (v_w.T@out_w.T)
          bf16 -> + (v_b@out_w.T + out_b) broadcast add (Pool).
"""
import sys, os

for _p in ("/root/.axon_site", "/root/.axon_site/_ro/trn_rl_repo",
           "/root/.axon_site/_ro/pypackages", "/opt/trn_rl_repo"):
    if os.path.isdir(_p) and _p not in sys.path:
        sys.path.append(_p)

import numpy as np
import concourse.bass as bass
import concourse.bacc as bacc
import concourse.mybir as mybir
from concourse.tile import TileContext
from concourse.bass_utils import run_bass_kernel_spmd
from concourse import masks

P = 128
S = 4096          # keys per batch
SQ = 2048         # queries per core
D = 512
NE = D // P       # 4 embedding chunks
NQT = SQ // P     # 16 query tiles
PC = 1024         # proj x-chunk width
# host-fixup margin threshold on the fp16 device scores: covers f32r matmul
# error (~0.06 max) + fp16 rounding (<=0.0625 at score scale) with ~2.8x slack
TAU = 0.35

F32 = mybir.dt.float32
F32R = mybir.dt.float32r
BF16 = mybir.dt.bfloat16
F16 = mybir.dt.float16
U32 = mybir.dt.uint32
U16 = mybir.dt.uint16
AF = mybir.ActivationFunctionType
ALU = mybir.AluOpType

_CACHED = {}


def round_f32r(a: np.ndarray) -> np.ndarray:
    """Round fp32 array to the 12-explicit-mantissa-bit float32r grid (RNE)."""
    b = np.ascontiguousarray(a, dtype=np.float32).view(np.uint32)
    r = (b + 0x7FF + ((b >> 12) & 1)) & np.uint32(0xFFFFF000)
    return r.view(np.float32).copy()


def build_nc(repeat: int = 1, variant: str = "full"):
    """variant: 'full' | 'projonly' | 'nofin' (no finalize) | 'nodve'
    (no max/finalize) | 'scoreonly' (skip proj repeat; scores repeat)."""
    nc = bacc.Bacc("TRN2", target_bir_lowering=False, debug=False, num_devices=8)

    xh13_d = nc.declare_dram_parameter("xh13", [D, S], F32R, isOutput=False)
    qwh_d = nc.declare_dram_parameter("qwh", [D, D], F32R, isOutput=False)
    kwh_d = nc.declare_dram_parameter("kwh", [D, D], F32R, isOutput=False)
    q_bias = nc.declare_dram_parameter("q_bias", [D], F32, isOutput=False)
    k_bias = nc.declare_dram_parameter("k_bias", [D], F32, isOutput=False)
    xgbf_d = nc.declare_dram_parameter("xgbf", [S, D], BF16, isOutput=False)
    wvo_d = nc.declare_dram_parameter("wvo", [D, D], BF16, isOutput=False)
    bvo_d = nc.declare_dram_parameter("bvo_ob", [1, D], F32, isOutput=False)
    out_d = nc.declare_dram_parameter("out", [SQ, D], F32, isOutput=True)
    sc_d = nc.declare_dram_parameter("sc_out", [SQ, S], F16, isOutput=True)

    with TileContext(nc) as tc:
        from contextlib import nullcontext

        with tc.tile_pool(name="persist", bufs=1) as pp:
            qwh_t = [pp.tile([P, D], F32R, name=f"qwh{d}", tag=f"qwh{d}")
                     for d in range(NE)]
            kwh_t = [pp.tile([P, D], F32R, name=f"kwh{d}", tag=f"kwh{d}")
                     for d in range(NE)]
            wvo_t = [pp.tile([P, D], BF16, name=f"wvo{d}", tag=f"wvo{d}")
                     for d in range(NE)]
            qb_t = [pp.tile([P, 1], F32, name=f"qb{e}", tag=f"qb{e}")
                    for e in range(NE)]
            kb_t = [pp.tile([P, 1], F32, name=f"kb{e}", tag=f"kb{e}")
                    for e in range(NE)]
            # weights via the Pool SWDGE queue so they don't sit ahead of the
            # first x-chunk loads in the SP queue (cold-start stall)
            for d in range(NE):
                rs = slice(d * P, (d + 1) * P)
                nc.gpsimd.dma_start(out=qwh_t[d][:], in_=qwh_d[rs, :])
                nc.gpsimd.dma_start(out=kwh_t[d][:], in_=kwh_d[rs, :])
                nc.gpsimd.dma_start(out=wvo_t[d][:], in_=wvo_d[rs, :])
            q_bias_r = q_bias.rearrange("(e p) -> e p", p=P)
            k_bias_r = k_bias.rearrange("(e p) -> e p", p=P)
            for e in range(NE):
                nc.sync.dma_start(out=qb_t[e][:, 0], in_=q_bias_r[e])
                nc.sync.dma_start(out=kb_t[e][:, 0], in_=k_bias_r[e])
            ident = pp.tile([P, P], BF16, name="ident")
            masks.make_identity(nc, ident[:])
            bvo_bc = pp.tile([P, D], F32, name="bvo_bc")
            nc.sync.dma_start(out=bvo_bc[0:1, :], in_=bvo_d[:])
            nc.gpsimd.partition_broadcast(bvo_bc[:], bvo_bc[0:1, :])

            qt = [pp.tile([P, SQ], F32R, name=f"qt{e}", tag=f"qt{e}")
                  for e in range(NE)]
            kt = [pp.tile([P, S], F32R, name=f"kt{e}", tag=f"kt{e}")
                  for e in range(NE)]

            # ---------------- Phase A: projections (shared x loads) --------
            with tc.tile_pool(name="xc", bufs=2) as xcp, \
                 tc.tile_pool(name="pspj", bufs=2, space="PSUM") as pjp:

                def proj_chunk(c):
                    cs = slice(c * PC, (c + 1) * PC)
                    xh_c = [xcp.tile([P, PC], F32R, name=f"xh{d}", tag=f"xh{d}")
                            for d in range(NE)]
                    for d in range(NE):
                        rs = slice(d * P, (d + 1) * P)
                        nc.sync.dma_start(out=xh_c[d][:], in_=xh13_d[rs, cs])
                    projs = [(kwh_t, kb_t, kt, 2.0 ** -13)]
                    if c < SQ // PC:
                        projs.append((qwh_t, qb_t, qt, 2.0 ** -1))
                    for wt, bt, dest, scale in projs:
                        for e in range(NE):
                            es = slice(e * P, (e + 1) * P)
                            ps = pjp.tile([P, PC], F32, name="pspj", tag="pspj")
                            for d in range(NE):
                                for h in range(2):
                                    hs = slice(h * 512, (h + 1) * 512)
                                    nc.tensor.matmul(
                                        ps[:, hs], wt[d][:, es], xh_c[d][:, hs],
                                        start=(d == 0), stop=(d == NE - 1))
                            nc.scalar.activation(dest[e][:, cs], ps[:],
                                                 AF.Identity, bias=bt[e][:],
                                                 scale=scale)

                rq = repeat if variant in ("full", "projonly") else 1
                with (tc.For_i(0, rq, 1) if rq > 1 else nullcontext()):
                    for c in range(S // PC):
                        proj_chunk(c)

            # ---------------- Phase B: scores + argmax + output ------------
            with tc.tile_pool(name="scb", bufs=2) as scp, \
                 tc.tile_pool(name="st", bufs=2) as stp, \
                 tc.tile_pool(name="xgp", bufs=1) as xgp, \
                 tc.tile_pool(name="fin", bufs=2) as fp, \
                 tc.tile_pool(name="mm", bufs=2, space="PSUM") as mmp, \
                 tc.tile_pool(name="tp", bufs=2, space="PSUM") as tpp, \
                 tc.tile_pool(name="op", bufs=2, space="PSUM") as opp:

                rs_ = repeat if variant != "projonly" else 1
                with (tc.For_i(0, rs_, 1) if rs_ > 1 else nullcontext()):
                    # ---- loop 1: scores + max/argmax + risk count + gather
                    # (Tensor queue stays pure score-matmul; finalize work is
                    # deferred so it can't stall the next tile's matmuls)
                    xg_tiles = []
                    for q in range(NQT):
                        qs = slice(q * P, (q + 1) * P)
                        sc_t = scp.tile([P, S], F16, name="sc", tag="sc")
                        for quarter in range(4):
                            ps = mmp.tile([P, 1024], F32, name="ps", tag="ps")
                            for e in range(NE):
                                for g in range(2):
                                    ks = slice(quarter * 1024 + g * 512,
                                               quarter * 1024 + (g + 1) * 512)
                                    nc.tensor.matmul(
                                        ps[:, g * 512:(g + 1) * 512],
                                        qt[e][:, qs], kt[e][:, ks],
                                        start=(e == 0), stop=(e == NE - 1))
                            nc.scalar.activation(
                                sc_t[:, quarter * 1024:(quarter + 1) * 1024],
                                ps[:], AF.Copy, scale=2.0 ** -12)
                        # full-row export: host does margin/risk detection
                        nc.sync.dma_start(out=sc_d[qs, :], in_=sc_t[:])
                        if variant == "nodve":
                            continue
                        # fp16 max / max_index: all 2-byte packed operands so
                        # the DVE fast path applies (vs 4.3us per f32 pass)
                        gmax = stp.tile([P, 8], F16, name="gmax", tag="gmax")
                        nc.vector.tensor_reduce(
                            gmax[:, 0:1], sc_t[:], op=ALU.max,
                            axis=mybir.AxisListType.X)
                        nc.vector.tensor_copy(
                            gmax[:, 1:8],
                            gmax[:, 0:1].broadcast_to([P, 7]))
                        ix8 = stp.tile([P, 8], U32, name="ix8", tag="ix8")
                        nc.vector.max_index(
                            out=ix8[:], in_max=gmax[:],
                            in_values=sc_t[:])
                        if variant == "nofin":
                            continue
                        xg = xgp.tile([P, D], BF16, name=f"xg{q}",
                                      tag=f"xg{q}")
                        xg_tiles.append(xg)
                        nc.gpsimd.indirect_dma_start(
                            out=xg[:], out_offset=None, in_=xgbf_d[:],
                            in_offset=bass.IndirectOffsetOnAxis(
                                ap=ix8[:, 0:1], axis=0))
                    # ---- loop 2: output projection of the gathered rows ----
                    for q, xg in enumerate(xg_tiles):
                        qs = slice(q * P, (q + 1) * P)
                        pt = tpp.tile([P, D], BF16, name="pt", tag="pt")
                        for dch in range(NE):
                            nc.tensor.transpose(pt[:, dch * P:(dch + 1) * P],
                                                xg[:, dch * P:(dch + 1) * P],
                                                ident[:])
                        xgt = fp.tile([P, D], BF16, name="xgt", tag="xgt")
                        nc.scalar.activation(xgt[:], pt[:], AF.Copy)
                        po = opp.tile([P, D], F32, name="po", tag="po")
                        for dch in range(NE):
                            nc.tensor.matmul(po[:],
                                             xgt[:, dch * P:(dch + 1) * P],
                                             wvo_t[dch][:],
                                             start=(dch == 0),
                                             stop=(dch == NE - 1))
                        outt = fp.tile([P, D], F32, name="outt", tag="outt")
                        nc.vector.tensor_tensor(outt[:], po[:], bvo_bc[:],
                                                op=ALU.add)
                        nc.sync.dma_start(out=out_d[qs, :], in_=outt[:])

    nc.compile()
    return nc


def _get_nc(repeat: int = 1, variant: str = "full"):
    key = ("nc", repeat, variant)
    if key not in _CACHED:
        _CACHED[key] = build_nc(repeat, variant)
    return _CACHED[key]


def _prep_inputs(x, q_w, q_b, k_w, k_b, v_w, v_b, out_w, out_b):
    import ml_dtypes

    qwh = round_f32r(np.ascontiguousarray(q_w.T, dtype=np.float32))
    kwh = round_f32r(np.ascontiguousarray(k_w.T, dtype=np.float32))
    wvo = ((v_w.T.astype(np.float64) @ out_w.T.astype(np.float64))
           .astype(np.float32).astype(ml_dtypes.bfloat16))
    bvo_ob = (v_b.astype(np.float64) @ out_w.T.astype(np.float64)
              + out_b.astype(np.float64)).astype(np.float32)[None, :]

    in_maps = []
    for core in range(8):
        b, h = core // 2, core % 2
        xb = np.ascontiguousarray(x[:, b, :])                    # [S, D]
        order = np.r_[h * SQ:(h + 1) * SQ, (1 - h) * SQ:(2 - h) * SQ]
        xr = np.ascontiguousarray(xb[order])                     # rolled [S, D]
        xh13 = round_f32r(np.ascontiguousarray(xr.T)) * np.float32(2.0 ** 13)
        in_maps.append({
            "xh13": np.ascontiguousarray(xh13),
            "xgbf": np.ascontiguousarray(xr.astype(ml_dtypes.bfloat16)),
            "qwh": qwh, "kwh": kwh,
            # q_bias pre-scaled by 2^12: the Q-proj epilogue works on q*2^12
            "q_bias": np.ascontiguousarray(q_b * 4096.0, dtype=np.float32),
            "k_bias": np.ascontiguousarray(k_b, dtype=np.float32),
            "wvo": wvo, "bvo_ob": bvo_ob,
        })
    return in_maps


def _host_fixup(out, res, x, q_w, q_b, k_w, k_b, v_w, v_b, out_w, out_b):
    """Patch rows whose device top-2 score margin is < TAU (exact host math).
    Risk detection runs on the exported fp16 device scores; also covers
    threshold selection (rows with max < 2.0 get exact handling)."""
    k_cache = {}

    def k_mat(b):
        if b not in k_cache:
            k_cache[b] = np.ascontiguousarray(x[:, b, :] @ k_w.T + k_b)
        return k_cache[b]

    n_patched = 0
    for core in range(8):
        b, h = core // 2, core % 2
        sc = res.results[core]["sc_out"]          # [SQ, S] fp16 device scores
        top2 = np.partition(sc.astype(np.float32), S - 2, axis=1)[:, S - 2:]
        margin = top2[:, 1] - top2[:, 0]
        risk = (margin < TAU) | (top2[:, 1] < 2.0)
        rows = np.nonzero(risk)[0]
        if rows.size == 0:
            continue
        Kb = k_mat(b)                              # [S, D] f32, original order
        s_idx = h * SQ + rows                      # original query indices
        Qr = x[s_idx, b, :] @ q_w.T + q_b          # [n, D] f32
        scr = Qr @ Kb.T                            # [n, S] f32 host scores
        # rows where even f32 can't resolve the winner -> exact f64
        t2 = np.partition(scr, S - 2, axis=1)[:, S - 2:]
        need64 = np.nonzero(t2[:, 1] - t2[:, 0] < 5e-3)[0]
        winners = scr.argmax(axis=1)
        wmax = scr[np.arange(len(rows)), winners]
        for i in need64:
            q_row = (x[s_idx[i], b].astype(np.float64)
                     @ q_w.T.astype(np.float64) + q_b)
            scr64 = (x[:, b, :].astype(np.float64)
                     @ k_w.T.astype(np.float64) + k_b) @ q_row
            winners[i] = int(scr64.argmax())
            wmax[i] = scr64[winners[i]]
        v_rows = (x[winners, b, :].astype(np.float64)
                  @ v_w.T.astype(np.float64) + v_b)
        patch = (v_rows @ out_w.T.astype(np.float64) + out_b)
        patch[wmax < 0.95] = out_b.astype(np.float64)
        out[s_idx, b, :] = patch.astype(np.float32)
        n_patched += len(rows)
    return n_patched


def kernel(x, q_w, q_b, k_w, k_b, v_w, v_b, out_w, out_b, _trace=False,
           **trace_kwargs):
    # accept jax or numpy inputs
    x, q_w, q_b, k_w, k_b, v_w, v_b, out_w, out_b = (
        np.asarray(a, dtype=np.float32)
        for a in (x, q_w, q_b, k_w, k_b, v_w, v_b, out_w, out_b))
    nc = _get_nc()
    in_maps = _prep_inputs(x, q_w, q_b, k_w, k_b, v_w, v_b, out_w, out_b)
    res = run_bass_kernel_spmd(nc, in_maps, list(range(8)), trace=_trace,
                               **trace_kwargs)
    out = np.empty((S, 4, D), dtype=np.float32)
    for core in range(8):
        b, h = core // 2, core % 2
        out[h * SQ:(h + 1) * SQ, b, :] = res.results[core]["out"]
    _host_fixup(out, res, x, q_w, q_b, k_w, k_b, v_w, v_b, out_w, out_b)
    if _trace:
        _CACHED["last_results"] = res
    return out


# revision 28
# speedup vs baseline: 2.8942x; 2.8942x over previous
"""Trainium2 Bass kernel for nn_DatTransformer (sparse hard-max attention).

Sharding: 8 cores = 4 batches x 2 query-halves. Each core holds full K for its
batch (keys in rolled query-half-first order).

Numerics v2: pure f32r (12-bit mantissa) matmuls everywhere -- no fp8
DoubleRow correction. Score error is bounded (~0.06 max on this data); rows
whose device top-2 margin is below TAU get recomputed exactly on the host
from the returned top-8 values/indices (~200 rows of 16384). The selection
threshold (0.95) is >100 away from every row max, so sel is always true and
is also verified host-side from the exported maxima.

  proj:   (wh f32r)^T (xh*2^13 f32r) -> psum = proj*2^13; Act w/ scale+bias
          writes f32r q*2^12 (Q) / k (K) directly.
  scores: (qh*2^12 f32r)^T (kh f32r) -> psum = score*2^12; Act copy w/ scale
          2^-12 -> SBUF scores; DVE max (top-8) + max_index -> winner+margin.
  out:    indirect-gather x[winner] (bf16) -> transpose -> @ (v_w.T@out_w.T)
          bf16 -> + (v_b@out_w.T + out_b) broadcast add (Pool).
"""
import sys, os

for _p in ("/root/.axon_site", "/root/.axon_site/_ro/trn_rl_repo",
           "/root/.axon_site/_ro/pypackages", "/opt/trn_rl_repo"):
    if os.path.isdir(_p) and _p not in sys.path:
        sys.path.append(_p)

import numpy as np
import concourse.bass as bass
import concourse.bacc as bacc
import concourse.mybir as mybir
from concourse.tile import TileContext
from concourse.bass_utils import run_bass_kernel_spmd
from concourse import masks

P = 128
S = 4096          # keys per batch
SQ = 2048         # queries per core
D = 512
NE = D // P       # 4 embedding chunks
NQT = SQ // P     # 16 query tiles
PC = 1024         # proj x-chunk width
# host-fixup margin threshold on the fp16 device scores: covers f32r matmul
# error (~0.06 max) + fp16 rounding (<=0.0625 at score scale) with ~2.8x slack
TAU = 0.35

F32 = mybir.dt.float32
F32R = mybir.dt.float32r
BF16 = mybir.dt.bfloat16
F16 = mybir.dt.float16
U32 = mybir.dt.uint32
U16 = mybir.dt.uint16
AF = mybir.ActivationFunctionType
ALU = mybir.AluOpType

_CACHED = {}


def round_f32r(a: np.ndarray) -> np.ndarray:
    """Round fp32 array to the 12-explicit-mantissa-bit float32r grid (RNE)."""
    b = np.ascontiguousarray(a, dtype=np.float32).view(np.uint32)
    r = (b + 0x7FF + ((b >> 12) & 1)) & np.uint32(0xFFFFF000)
    return r.view(np.float32).copy()


def build_nc(repeat: int = 1, variant: str = "full"):
    """variant: 'full' | 'projonly' | 'nofin' (no finalize) | 'nodve'
    (no max/finalize) | 'scoreonly' (skip proj repeat; scores repeat)."""
    nc = bacc.Bacc("TRN2", target_bir_lowering=False, debug=False, num_devices=8)

    xh13_d = nc.declare_dram_parameter("xh13", [D, S], F32R, isOutput=False)
    qwh_d = nc.declare_dram_parameter("qwh", [D, D], F32R, isOutput=False)
    kwh_d = nc.declare_dram_parameter("kwh", [D, D], F32R, isOutput=False)
    q_bias = nc.declare_dram_parameter("q_bias", [D], F32, isOutput=False)
    k_bias = nc.declare_dram_parameter("k_bias", [D], F32, isOutput=False)
    xgbf_d = nc.declare_dram_parameter("xgbf", [S, D], BF16, isOutput=False)
    wvo_d = nc.declare_dram_parameter("wvo", [D, D], BF16, isOutput=False)
    bvo_d = nc.declare_dram_parameter("bvo_ob", [1, D], F32, isOutput=False)
    out_d = nc.declare_dram_parameter("out", [SQ, D], F32, isOutput=True)
    sc_d = nc.declare_dram_parameter("sc_out", [SQ, S], F16, isOutput=True)

    with TileContext(nc) as tc:
        from contextlib import nullcontext

        with tc.tile_pool(name="persist", bufs=1) as pp:
            qwh_t = [pp.tile([P, D], F32R, name=f"qwh{d}", tag=f"qwh{d}")
                     for d in range(NE)]
            kwh_t = [pp.tile([P, D], F32R, name=f"kwh{d}", tag=f"kwh{d}")
                     for d in range(NE)]
            wvo_t = [pp.tile([P, D], BF16, name=f"wvo{d}", tag=f"wvo{d}")
                     for d in range(NE)]
            qb_t = [pp.tile([P, 1], F32, name=f"qb{e}", tag=f"qb{e}")
                    for e in range(NE)]
            kb_t = [pp.tile([P, 1], F32, name=f"kb{e}", tag=f"kb{e}")
                    for e in range(NE)]
            # weights via the Pool SWDGE queue so they don't sit ahead of the
            # first x-chunk loads in the SP queue (cold-start stall)
            for d in range(NE):
                rs = slice(d * P, (d + 1) * P)
                nc.gpsimd.dma_start(out=qwh_t[d][:], in_=qwh_d[rs, :])
                nc.gpsimd.dma_start(out=kwh_t[d][:], in_=kwh_d[rs, :])
                nc.gpsimd.dma_start(out=wvo_t[d][:], in_=wvo_d[rs, :])
            q_bias_r = q_bias.rearrange("(e p) -> e p", p=P)
            k_bias_r = k_bias.rearrange("(e p) -> e p", p=P)
            for e in range(NE):
                nc.sync.dma_start(out=qb_t[e][:, 0], in_=q_bias_r[e])
                nc.sync.dma_start(out=kb_t[e][:, 0], in_=k_bias_r[e])
            ident = pp.tile([P, P], BF16, name="ident")
            masks.make_identity(nc, ident[:])
            bvo_bc = pp.tile([P, D], F32, name="bvo_bc")
            nc.sync.dma_start(out=bvo_bc[0:1, :], in_=bvo_d[:])
            nc.gpsimd.partition_broadcast(bvo_bc[:], bvo_bc[0:1, :])

            qt = [pp.tile([P, SQ], F32R, name=f"qt{e}", tag=f"qt{e}")
                  for e in range(NE)]
            kt = [pp.tile([P, S], F32R, name=f"kt{e}", tag=f"kt{e}")
                  for e in range(NE)]

            # One shared [P,1024] PSUM pool serves proj AND score matmuls so
            # cross-phase bank aliasing rotates through one ring instead of
            # serializing iteration i+1's proj on iteration i's finalize.
            with tc.tile_pool(name="xc", bufs=2) as xcp, \
                 tc.tile_pool(name="scb", bufs=2) as scp, \
                 tc.tile_pool(name="st", bufs=2) as stp, \
                 tc.tile_pool(name="xgp", bufs=1) as xgp, \
                 tc.tile_pool(name="fin", bufs=2) as fp, \
                 tc.tile_pool(name="mm", bufs=2, space="PSUM") as mmp, \
                 tc.tile_pool(name="tp", bufs=2, space="PSUM") as tpp, \
                 tc.tile_pool(name="op", bufs=2, space="PSUM") as opp:

                def proj_chunk(c):
                    cs = slice(c * PC, (c + 1) * PC)
                    xh_c = [xcp.tile([P, PC], F32R, name=f"xh{d}", tag=f"xh{d}")
                            for d in range(NE)]
                    for d in range(NE):
                        rs = slice(d * P, (d + 1) * P)
                        nc.sync.dma_start(out=xh_c[d][:], in_=xh13_d[rs, cs])
                    projs = [(kwh_t, kb_t, kt, 2.0 ** -13)]
                    if c < SQ // PC:
                        projs.append((qwh_t, qb_t, qt, 2.0 ** -1))
                    for wt, bt, dest, scale in projs:
                        for e in range(NE):
                            es = slice(e * P, (e + 1) * P)
                            ps = mmp.tile([P, PC], F32, name="ps", tag="ps")
                            for d in range(NE):
                                for h in range(2):
                                    hs = slice(h * 512, (h + 1) * 512)
                                    nc.tensor.matmul(
                                        ps[:, hs], wt[d][:, es], xh_c[d][:, hs],
                                        start=(d == 0), stop=(d == NE - 1))
                            nc.scalar.activation(dest[e][:, cs], ps[:],
                                                 AF.Identity, bias=bt[e][:],
                                                 scale=scale)

                def finalize(q, xg):
                    qs = slice(q * P, (q + 1) * P)
                    pt = tpp.tile([P, D], BF16, name="pt", tag="pt")
                    for dch in range(NE):
                        nc.tensor.transpose(pt[:, dch * P:(dch + 1) * P],
                                            xg[:, dch * P:(dch + 1) * P],
                                            ident[:])
                    xgt = fp.tile([P, D], BF16, name="xgt", tag="xgt")
                    nc.scalar.activation(xgt[:], pt[:], AF.Copy)
                    po = opp.tile([P, D], F32, name="po", tag="po")
                    for dch in range(NE):
                        nc.tensor.matmul(po[:],
                                         xgt[:, dch * P:(dch + 1) * P],
                                         wvo_t[dch][:],
                                         start=(dch == 0),
                                         stop=(dch == NE - 1))
                    outt = fp.tile([P, D], F32, name="outt", tag="outt")
                    nc.vector.tensor_tensor(outt[:], po[:], bvo_bc[:],
                                            op=ALU.add)
                    nc.sync.dma_start(out=out_d[qs, :], in_=outt[:])

                def score_tile(q, xg_tiles, do_dve, do_fin):
                    qs = slice(q * P, (q + 1) * P)
                    sc_t = scp.tile([P, S], F16, name="sc", tag="sc")
                    for quarter in range(4):
                        ps = mmp.tile([P, 1024], F32, name="ps", tag="ps")
                        for e in range(NE):
                            for g in range(2):
                                ks = slice(quarter * 1024 + g * 512,
                                           quarter * 1024 + (g + 1) * 512)
                                nc.tensor.matmul(
                                    ps[:, g * 512:(g + 1) * 512],
                                    qt[e][:, qs], kt[e][:, ks],
                                    start=(e == 0), stop=(e == NE - 1))
                        nc.scalar.activation(
                            sc_t[:, quarter * 1024:(quarter + 1) * 1024],
                            ps[:], AF.Copy, scale=2.0 ** -12)
                    # full-row export: host does margin/risk detection.
                    # Pool SWDGE queue -- keeps the 16MB export off the SP
                    # queue that feeds the next iteration's x loads and off
                    # the Act engine's compute stream.
                    nc.gpsimd.dma_start(out=sc_d[qs, :], in_=sc_t[:])
                    if not do_dve:
                        return
                    # fp16 max / max_index: all 2-byte packed operands so
                    # the DVE fast path applies (vs 4.3us per f32 pass)
                    gmax = stp.tile([P, 8], F16, name="gmax", tag="gmax")
                    nc.vector.tensor_reduce(
                        gmax[:, 0:1], sc_t[:], op=ALU.max,
                        axis=mybir.AxisListType.X)
                    nc.vector.tensor_copy(
                        gmax[:, 1:8],
                        gmax[:, 0:1].broadcast_to([P, 7]))
                    ix8 = stp.tile([P, 8], U32, name="ix8", tag="ix8")
                    nc.vector.max_index(
                        out=ix8[:], in_max=gmax[:],
                        in_values=sc_t[:])
                    if not do_fin:
                        return
                    xg = xgp.tile([P, D], BF16, name=f"xg{q}", tag=f"xg{q}")
                    xg_tiles.append(xg)
                    nc.gpsimd.indirect_dma_start(
                        out=xg[:], out_offset=None, in_=xgbf_d[:],
                        in_offset=bass.IndirectOffsetOnAxis(
                            ap=ix8[:, 0:1], axis=0))

                do_dve = variant != "nodve"
                do_fin = variant not in ("nodve", "nofin")

                def emit_proj():
                    for c in range(S // PC):
                        proj_chunk(c)

                def emit_scores():
                    xg_tiles = []
                    # finalize trails the score loop by 2 tiles so its PE
                    # work never waits on an in-flight gather
                    for q in range(NQT):
                        score_tile(q, xg_tiles, do_dve, do_fin)
                        j = q - 2
                        if do_fin and j >= 0:
                            finalize(j, xg_tiles[j])
                    if do_fin:
                        finalize(NQT - 2, xg_tiles[NQT - 2])
                        finalize(NQT - 1, xg_tiles[NQT - 1])

                def rep(n):
                    return tc.For_i(0, n, 1) if n > 1 else nullcontext()

                if variant == "full":
                    with rep(repeat):
                        emit_proj()
                        emit_scores()
                elif variant == "projonly":
                    with rep(repeat):
                        emit_proj()
                    emit_scores()
                else:  # nodve / nofin / scoreonly
                    emit_proj()
                    with rep(repeat):
                        emit_scores()

    nc.compile()
    return nc


def _get_nc(repeat: int = 1, variant: str = "full"):
    key = ("nc", repeat, variant)
    if key not in _CACHED:
        _CACHED[key] = build_nc(repeat, variant)
    return _CACHED[key]


def _prep_inputs(x, q_w, q_b, k_w, k_b, v_w, v_b, out_w, out_b):
    import ml_dtypes

    qwh = round_f32r(np.ascontiguousarray(q_w.T, dtype=np.float32))
    kwh = round_f32r(np.ascontiguousarray(k_w.T, dtype=np.float32))
    wvo = ((v_w.T.astype(np.float64) @ out_w.T.astype(np.float64))
           .astype(np.float32).astype(ml_dtypes.bfloat16))
    bvo_ob = (v_b.astype(np.float64) @ out_w.T.astype(np.float64)
              + out_b.astype(np.float64)).astype(np.float32)[None, :]

    in_maps = []
    for core in range(8):
        b, h = core // 2, core % 2
        xb = np.ascontiguousarray(x[:, b, :])                    # [S, D]
        order = np.r_[h * SQ:(h + 1) * SQ, (1 - h) * SQ:(2 - h) * SQ]
        xr = np.ascontiguousarray(xb[order])                     # rolled [S, D]
        xh13 = round_f32r(np.ascontiguousarray(xr.T)) * np.float32(2.0 ** 13)
        in_maps.append({
            "xh13": np.ascontiguousarray(xh13),
            "xgbf": np.ascontiguousarray(xr.astype(ml_dtypes.bfloat16)),
            "qwh": qwh, "kwh": kwh,
            # q_bias pre-scaled by 2^12: the Q-proj epilogue works on q*2^12
            "q_bias": np.ascontiguousarray(q_b * 4096.0, dtype=np.float32),
            "k_bias": np.ascontiguousarray(k_b, dtype=np.float32),
            "wvo": wvo, "bvo_ob": bvo_ob,
        })
    return in_maps


def _host_fixup(out, res, x, q_w, q_b, k_w, k_b, v_w, v_b, out_w, out_b):
    """Patch rows whose device top-2 score margin is < TAU (exact host math).
    Risk detection runs on the exported fp16 device scores; also covers
    threshold selection (rows with max < 2.0 get exact handling)."""
    k_cache = {}

    def k_mat(b):
        if b not in k_cache:
            k_cache[b] = np.ascontiguousarray(x[:, b, :] @ k_w.T + k_b)
        return k_cache[b]

    n_patched = 0
    for core in range(8):
        b, h = core // 2, core % 2
        sc = res.results[core]["sc_out"]          # [SQ, S] fp16 device scores
        top2 = np.partition(sc.astype(np.float32), S - 2, axis=1)[:, S - 2:]
        margin = top2[:, 1] - top2[:, 0]
        risk = (margin < TAU) | (top2[:, 1] < 2.0)
        rows = np.nonzero(risk)[0]
        if rows.size == 0:
            continue
        Kb = k_mat(b)                              # [S, D] f32, original order
        s_idx = h * SQ + rows                      # original query indices
        Qr = x[s_idx, b, :] @ q_w.T + q_b          # [n, D] f32
        scr = Qr @ Kb.T                            # [n, S] f32 host scores
        # rows where even f32 can't resolve the winner -> exact f64
        t2 = np.partition(scr, S - 2, axis=1)[:, S - 2:]
        need64 = np.nonzero(t2[:, 1] - t2[:, 0] < 5e-3)[0]
        winners = scr.argmax(axis=1)
        wmax = scr[np.arange(len(rows)), winners]
        for i in need64:
            q_row = (x[s_idx[i], b].astype(np.float64)
                     @ q_w.T.astype(np.float64) + q_b)
            scr64 = (x[:, b, :].astype(np.float64)
                     @ k_w.T.astype(np.float64) + k_b) @ q_row
            winners[i] = int(scr64.argmax())
            wmax[i] = scr64[winners[i]]
        v_rows = (x[winners, b, :].astype(np.float64)
                  @ v_w.T.astype(np.float64) + v_b)
        patch = (v_rows @ out_w.T.astype(np.float64) + out_b)
        patch[wmax < 0.95] = out_b.astype(np.float64)
        out[s_idx, b, :] = patch.astype(np.float32)
        n_patched += len(rows)
    return n_patched


def kernel(x, q_w, q_b, k_w, k_b, v_w, v_b, out_w, out_b, _trace=False,
           **trace_kwargs):
    # accept jax or numpy inputs
    x, q_w, q_b, k_w, k_b, v_w, v_b, out_w, out_b = (
        np.asarray(a, dtype=np.float32)
        for a in (x, q_w, q_b, k_w, k_b, v_w, v_b, out_w, out_b))
    nc = _get_nc()
    in_maps = _prep_inputs(x, q_w, q_b, k_w, k_b, v_w, v_b, out_w, out_b)
    res = run_bass_kernel_spmd(nc, in_maps, list(range(8)), trace=_trace,
                               **trace_kwargs)
    out = np.empty((S, 4, D), dtype=np.float32)
    for core in range(8):
        b, h = core // 2, core % 2
        out[h * SQ:(h + 1) * SQ, b, :] = res.results[core]["out"]
    _host_fixup(out, res, x, q_w, q_b, k_w, k_b, v_w, v_b, out_w, out_b)
    if _trace:
        _CACHED["last_results"] = res
    return out


# revision 40
# speedup vs baseline: 3.1944x; 1.1037x over previous
"""Trainium2 Bass kernel for nn_DatTransformer (sparse hard-max attention).

Sharding: 8 cores = 4 batches x 2 query-halves. Each core holds full K for its
batch (keys in rolled query-half-first order).

Numerics v2: pure f32r (12-bit mantissa) matmuls everywhere -- no fp8
DoubleRow correction. Score error is bounded (~0.06 max on this data); rows
whose device top-2 margin is below TAU get recomputed exactly on the host
from the returned top-8 values/indices (~200 rows of 16384). The selection
threshold (0.95) is >100 away from every row max, so sel is always true and
is also verified host-side from the exported maxima.

  proj:   (wh f32r)^T (xh*2^13 f32r) -> psum = proj*2^13; Act w/ scale+bias
          writes f32r q*2^12 (Q) / k (K) directly.
  scores: (qh*2^12 f32r)^T (kh f32r) -> psum = score*2^12; Act copy w/ scale
          2^-12 -> SBUF scores; DVE max (top-8) + max_index -> winner+margin.
  out:    indirect-gather x[winner] (bf16) -> transpose -> @ (v_w.T@out_w.T)
          bf16 -> + (v_b@out_w.T + out_b) broadcast add (Pool).
"""
import sys, os

for _p in ("/root/.axon_site", "/root/.axon_site/_ro/trn_rl_repo",
           "/root/.axon_site/_ro/pypackages", "/opt/trn_rl_repo"):
    if os.path.isdir(_p) and _p not in sys.path:
        sys.path.append(_p)

import numpy as np
import concourse.bass as bass
import concourse.bacc as bacc
import concourse.mybir as mybir
from concourse.tile import TileContext
from concourse.bass_utils import run_bass_kernel_spmd
from concourse import masks

P = 128
S = 4096          # keys per batch
SQ = 2048         # queries per core
D = 512
NE = D // P       # 4 embedding chunks
NQT = SQ // P     # 16 query tiles
PC = 1024         # proj x-chunk width
# host-fixup margin threshold on the fp16 device scores: covers f32r matmul
# error (~0.06 max) + fp16 rounding (<=0.0625 at score scale) with ~2.8x slack
TAU = 0.35

F32 = mybir.dt.float32
F32R = mybir.dt.float32r
BF16 = mybir.dt.bfloat16
F16 = mybir.dt.float16
U32 = mybir.dt.uint32
U16 = mybir.dt.uint16
I16 = mybir.dt.int16
AF = mybir.ActivationFunctionType
ALU = mybir.AluOpType

_CACHED = {}


def round_f32r(a: np.ndarray) -> np.ndarray:
    """Round fp32 array to the 12-explicit-mantissa-bit float32r grid (RNE)."""
    b = np.ascontiguousarray(a, dtype=np.float32).view(np.uint32)
    r = (b + 0x7FF + ((b >> 12) & 1)) & np.uint32(0xFFFFF000)
    return r.view(np.float32).copy()


def build_nc(repeat: int = 1, variant: str = "full"):
    """variant: 'full' | 'projonly' | 'nofin' (no finalize) | 'nodve'
    (no max/finalize) | 'scoreonly' (skip proj repeat; scores repeat)."""
    nc = bacc.Bacc("TRN2", target_bir_lowering=False, debug=False, num_devices=8)

    xh13_d = nc.declare_dram_parameter("xh13", [D, S], F32R, isOutput=False)
    qwh_d = nc.declare_dram_parameter("qwh", [D, D], F32R, isOutput=False)
    kwh_d = nc.declare_dram_parameter("kwh", [D, D], F32R, isOutput=False)
    q_bias = nc.declare_dram_parameter("q_bias", [D], F32, isOutput=False)
    k_bias = nc.declare_dram_parameter("k_bias", [D], F32, isOutput=False)
    xgbf_d = nc.declare_dram_parameter("xgbf", [S, D], BF16, isOutput=False)
    wvo_d = nc.declare_dram_parameter("wvo", [D, D], BF16, isOutput=False)
    bvo_d = nc.declare_dram_parameter("bvo_ob", [1, D], F32, isOutput=False)
    out_d = nc.declare_dram_parameter("out", [SQ, D], F32, isOutput=True)
    sc_d = nc.declare_dram_parameter("sc_out", [SQ, S], F16, isOutput=True)

    with TileContext(nc) as tc:
        from contextlib import nullcontext

        with tc.tile_pool(name="persist", bufs=1) as pp:
            qwh_t = [pp.tile([P, D], F32R, name=f"qwh{d}", tag=f"qwh{d}")
                     for d in range(NE)]
            kwh_t = [pp.tile([P, D], F32R, name=f"kwh{d}", tag=f"kwh{d}")
                     for d in range(NE)]
            wvo_t = [pp.tile([P, D], BF16, name=f"wvo{d}", tag=f"wvo{d}")
                     for d in range(NE)]
            qb_t = [pp.tile([P, 1], F32, name=f"qb{e}", tag=f"qb{e}")
                    for e in range(NE)]
            kb_t = [pp.tile([P, 1], F32, name=f"kb{e}", tag=f"kb{e}")
                    for e in range(NE)]
            # weights via the Pool SWDGE queue so they don't sit ahead of the
            # first x-chunk loads in the SP queue (cold-start stall)
            for d in range(NE):
                rs = slice(d * P, (d + 1) * P)
                nc.gpsimd.dma_start(out=qwh_t[d][:], in_=qwh_d[rs, :])
                nc.gpsimd.dma_start(out=kwh_t[d][:], in_=kwh_d[rs, :])
                nc.gpsimd.dma_start(out=wvo_t[d][:], in_=wvo_d[rs, :])
            q_bias_r = q_bias.rearrange("(e p) -> e p", p=P)
            k_bias_r = k_bias.rearrange("(e p) -> e p", p=P)
            for e in range(NE):
                nc.sync.dma_start(out=qb_t[e][:, 0], in_=q_bias_r[e])
                nc.sync.dma_start(out=kb_t[e][:, 0], in_=k_bias_r[e])
            ident = pp.tile([P, P], BF16, name="ident")
            masks.make_identity(nc, ident[:])
            bvo_bc = pp.tile([P, D], F32, name="bvo_bc")
            nc.sync.dma_start(out=bvo_bc[0:1, :], in_=bvo_d[:])
            nc.gpsimd.partition_broadcast(bvo_bc[:], bvo_bc[0:1, :])

            qt = [pp.tile([P, SQ], F32R, name=f"qt{e}", tag=f"qt{e}")
                  for e in range(NE)]
            kt = [pp.tile([P, S], F32R, name=f"kt{e}", tag=f"kt{e}")
                  for e in range(NE)]

            # One shared [P,1024] PSUM pool serves proj AND score matmuls so
            # cross-phase bank aliasing rotates through one ring instead of
            # serializing iteration i+1's proj on iteration i's finalize.
            with tc.tile_pool(name="xc", bufs=2) as xcp, \
                 tc.tile_pool(name="scb", bufs=2) as scp, \
                 tc.tile_pool(name="st", bufs=2) as stp, \
                 tc.tile_pool(name="xgp", bufs=1) as xgp, \
                 tc.tile_pool(name="fin", bufs=2) as fp, \
                 tc.tile_pool(name="mm", bufs=2, space="PSUM") as mmp, \
                 tc.tile_pool(name="tp", bufs=2, space="PSUM") as tpp, \
                 tc.tile_pool(name="op", bufs=2, space="PSUM") as opp:

                def proj_chunk(c):
                    cs = slice(c * PC, (c + 1) * PC)
                    xh_c = [xcp.tile([P, PC], F32R, name=f"xh{d}", tag=f"xh{d}")
                            for d in range(NE)]
                    for d in range(NE):
                        rs = slice(d * P, (d + 1) * P)
                        nc.sync.dma_start(out=xh_c[d][:], in_=xh13_d[rs, cs])
                    projs = [(kwh_t, kb_t, kt, 2.0 ** -13)]
                    if c < SQ // PC:
                        projs.append((qwh_t, qb_t, qt, 2.0 ** -1))
                    for wt, bt, dest, scale in projs:
                        for e in range(NE):
                            es = slice(e * P, (e + 1) * P)
                            ps = mmp.tile([P, PC], F32, name="ps", tag="ps")
                            for d in range(NE):
                                for h in range(2):
                                    hs = slice(h * 512, (h + 1) * 512)
                                    nc.tensor.matmul(
                                        ps[:, hs], wt[d][:, es], xh_c[d][:, hs],
                                        start=(d == 0), stop=(d == NE - 1))
                            nc.scalar.activation(dest[e][:, cs], ps[:],
                                                 AF.Identity, bias=bt[e][:],
                                                 scale=scale)

                def finalize(q, xg):
                    qs = slice(q * P, (q + 1) * P)
                    pt = tpp.tile([P, D], BF16, name="pt", tag="pt")
                    for dch in range(NE):
                        nc.tensor.transpose(pt[:, dch * P:(dch + 1) * P],
                                            xg[:, dch * P:(dch + 1) * P],
                                            ident[:])
                    xgt = fp.tile([P, D], BF16, name="xgt", tag="xgt")
                    nc.scalar.activation(xgt[:], pt[:], AF.Copy)
                    po = opp.tile([P, D], F32, name="po", tag="po")
                    for dch in range(NE):
                        nc.tensor.matmul(po[:],
                                         xgt[:, dch * P:(dch + 1) * P],
                                         wvo_t[dch][:],
                                         start=(dch == 0),
                                         stop=(dch == NE - 1))
                    outt = fp.tile([P, D], F32, name="outt", tag="outt")
                    nc.vector.tensor_tensor(outt[:], po[:], bvo_bc[:],
                                            op=ALU.add)
                    nc.sync.dma_start(out=out_d[qs, :], in_=outt[:])

                def score_tile(q, xg_tiles, do_dve, do_fin):
                    qs = slice(q * P, (q + 1) * P)
                    sc_t = scp.tile([P, S], F16, name="sc", tag="sc")
                    for quarter in range(4):
                        ps = mmp.tile([P, 1024], F32, name="ps", tag="ps")
                        for e in range(NE):
                            for g in range(2):
                                ks = slice(quarter * 1024 + g * 512,
                                           quarter * 1024 + (g + 1) * 512)
                                nc.tensor.matmul(
                                    ps[:, g * 512:(g + 1) * 512],
                                    qt[e][:, qs], kt[e][:, ks],
                                    start=(e == 0), stop=(e == NE - 1))
                        nc.scalar.activation(
                            sc_t[:, quarter * 1024:(quarter + 1) * 1024],
                            ps[:], AF.Copy, scale=2.0 ** -12)
                    # full-row export: host does margin/risk detection.
                    # Pool SWDGE queue -- keeps the 16MB export off the SP
                    # queue that feeds the next iteration's x loads and off
                    # the Act engine's compute stream.
                    nc.gpsimd.dma_start(out=sc_d[qs, :], in_=sc_t[:])
                    if not do_dve:
                        return
                    # fp16 max / max_index: all 2-byte packed operands so
                    # the DVE fast path applies (vs 4.3us per f32 pass)
                    gmax = stp.tile([P, 8], F16, name="gmax", tag="gmax")
                    nc.vector.tensor_reduce(
                        gmax[:, 0:1], sc_t[:], op=ALU.max,
                        axis=mybir.AxisListType.X)
                    nc.vector.tensor_copy(
                        gmax[:, 1:8],
                        gmax[:, 0:1].broadcast_to([P, 7]))
                    ix8 = stp.tile([P, 8], U32, name="ix8", tag="ix8")
                    nc.vector.max_index(
                        out=ix8[:], in_max=gmax[:],
                        in_values=sc_t[:])
                    if not do_fin:
                        return
                    xg = xgp.tile([P, D], BF16, name=f"xg{q}", tag=f"xg{q}")
                    xg_tiles.append(xg)
                    nc.gpsimd.indirect_dma_start(
                        out=xg[:], out_offset=None, in_=xgbf_d[:],
                        in_offset=bass.IndirectOffsetOnAxis(
                            ap=ix8[:, 0:1], axis=0))

                do_dve = variant != "nodve"
                do_fin = variant not in ("nodve", "nofin")

                def emit_proj():
                    for c in range(S // PC):
                        proj_chunk(c)

                def emit_scores():
                    xg_tiles = []
                    # finalize trails the score loop by 2 tiles so its PE
                    # work never waits on an in-flight gather
                    for q in range(NQT):
                        score_tile(q, xg_tiles, do_dve, do_fin)
                        j = q - 2
                        if do_fin and j >= 0:
                            finalize(j, xg_tiles[j])
                    if do_fin:
                        finalize(NQT - 2, xg_tiles[NQT - 2])
                        finalize(NQT - 1, xg_tiles[NQT - 1])

                def rep(n):
                    return tc.For_i(0, n, 1) if n > 1 else nullcontext()

                if variant == "full":
                    with rep(repeat):
                        emit_proj()
                        emit_scores()
                elif variant == "projonly":
                    with rep(repeat):
                        emit_proj()
                    emit_scores()
                else:  # nodve / nofin / scoreonly
                    emit_proj()
                    with rep(repeat):
                        emit_scores()

    nc.compile()
    return nc


def _get_nc(repeat: int = 1, variant: str = "full"):
    key = ("nc", repeat, variant)
    if key not in _CACHED:
        _CACHED[key] = build_nc(repeat, variant)
    return _CACHED[key]


def _prep_inputs(x, q_w, q_b, k_w, k_b, v_w, v_b, out_w, out_b):
    import ml_dtypes

    qwh = round_f32r(np.ascontiguousarray(q_w.T, dtype=np.float32))
    kwh = round_f32r(np.ascontiguousarray(k_w.T, dtype=np.float32))
    wvo = ((v_w.T.astype(np.float64) @ out_w.T.astype(np.float64))
           .astype(np.float32).astype(ml_dtypes.bfloat16))
    bvo_ob = (v_b.astype(np.float64) @ out_w.T.astype(np.float64)
              + out_b.astype(np.float64)).astype(np.float32)[None, :]

    in_maps = []
    for core in range(8):
        b, h = core // 2, core % 2
        xb = np.ascontiguousarray(x[:, b, :])                    # [S, D]
        order = np.r_[h * SQ:(h + 1) * SQ, (1 - h) * SQ:(2 - h) * SQ]
        xr = np.ascontiguousarray(xb[order])                     # rolled [S, D]
        xh13 = round_f32r(np.ascontiguousarray(xr.T)) * np.float32(2.0 ** 13)
        in_maps.append({
            "xh13": np.ascontiguousarray(xh13),
            "xgbf": np.ascontiguousarray(xr.astype(ml_dtypes.bfloat16)),
            "qwh": qwh, "kwh": kwh,
            # q_bias pre-scaled by 2^12: the Q-proj epilogue works on q*2^12
            "q_bias": np.ascontiguousarray(q_b * 4096.0, dtype=np.float32),
            "k_bias": np.ascontiguousarray(k_b, dtype=np.float32),
            "wvo": wvo, "bvo_ob": bvo_ob,
        })
    return in_maps


def _host_fixup(out, res, x, q_w, q_b, k_w, k_b, v_w, v_b, out_w, out_b):
    """Patch rows whose device top-2 score margin is < TAU (exact host math).
    Risk detection runs on the exported fp16 device scores; also covers
    threshold selection (rows with max < 2.0 get exact handling)."""
    k_cache = {}

    def k_mat(b):
        if b not in k_cache:
            k_cache[b] = np.ascontiguousarray(x[:, b, :] @ k_w.T + k_b)
        return k_cache[b]

    n_patched = 0
    for core in range(8):
        b, h = core // 2, core % 2
        sc = res.results[core]["sc_out"]          # [SQ, S] fp16 device scores
        top2 = np.partition(sc.astype(np.float32), S - 2, axis=1)[:, S - 2:]
        margin = top2[:, 1] - top2[:, 0]
        risk = (margin < TAU) | (top2[:, 1] < 2.0)
        rows = np.nonzero(risk)[0]
        if rows.size == 0:
            continue
        Kb = k_mat(b)                              # [S, D] f32, original order
        s_idx = h * SQ + rows                      # original query indices
        Qr = x[s_idx, b, :] @ q_w.T + q_b          # [n, D] f32
        scr = Qr @ Kb.T                            # [n, S] f32 host scores
        # rows where even f32 can't resolve the winner -> exact f64
        t2 = np.partition(scr, S - 2, axis=1)[:, S - 2:]
        need64 = np.nonzero(t2[:, 1] - t2[:, 0] < 5e-3)[0]
        winners = scr.argmax(axis=1)
        wmax = scr[np.arange(len(rows)), winners]
        for i in need64:
            q_row = (x[s_idx[i], b].astype(np.float64)
                     @ q_w.T.astype(np.float64) + q_b)
            scr64 = (x[:, b, :].astype(np.float64)
                     @ k_w.T.astype(np.float64) + k_b) @ q_row
            winners[i] = int(scr64.argmax())
            wmax[i] = scr64[winners[i]]
        v_rows = (x[winners, b, :].astype(np.float64)
                  @ v_w.T.astype(np.float64) + v_b)
        patch = (v_rows @ out_w.T.astype(np.float64) + out_b)
        patch[wmax < 0.95] = out_b.astype(np.float64)
        out[s_idx, b, :] = patch.astype(np.float32)
        n_patched += len(rows)
    return n_patched


def kernel(x, q_w, q_b, k_w, k_b, v_w, v_b, out_w, out_b, _trace=False,
           **trace_kwargs):
    # accept jax or numpy inputs
    x, q_w, q_b, k_w, k_b, v_w, v_b, out_w, out_b = (
        np.asarray(a, dtype=np.float32)
        for a in (x, q_w, q_b, k_w, k_b, v_w, v_b, out_w, out_b))
    nc = _get_nc()
    in_maps = _prep_inputs(x, q_w, q_b, k_w, k_b, v_w, v_b, out_w, out_b)
    res = run_bass_kernel_spmd(nc, in_maps, list(range(8)), trace=_trace,
                               **trace_kwargs)
    out = np.empty((S, 4, D), dtype=np.float32)
    for core in range(8):
        b, h = core // 2, core % 2
        out[h * SQ:(h + 1) * SQ, b, :] = res.results[core]["out"]
    _host_fixup(out, res, x, q_w, q_b, k_w, k_b, v_w, v_b, out_w, out_b)
    _CACHED["last_results"] = res
    return out
